# revision 7
# baseline (speedup 1.0000x reference)
"""DiagLinear kernel for 8 TRN2 NeuronCores.

Computes y = x * weight + bias  (weight/bias broadcast over the batch dim).

Strategy: transpose x on the host to xT [IN_SIZE, BATCH] and shard xT's rows
(the in_size dim) across the 8 cores. With in_size on the SBUF partition
axis, weight/bias become per-partition scalars, so the whole elementwise
computation is a single fused DVE tensor_scalar op per tile:
    out = (x * w) + b          (fp32, 2x perf mode)
which keeps the kernel firmly DMA-bound (the memory roofline for this
problem: 2 x 16.78 MB of HBM traffic per core at ~358 GB/s ~= 94 us).

Each row of the per-core input is augmented on the host with 2 leading
columns holding that row's weight and bias values, so every SBUF tile is
self-contained: the fused op reads its per-partition scalars from columns
0/1 of the tile it just loaded. The kernel is raw Bass (no Tile) with a
fully static schedule: 4 tiles of [128, 2+8192], loads and stores split
across the two HWDGE rings (SP and ACT sequencers), DVE compute chained
behind each load via semaphores.
"""

import numpy as np

import concourse.bass as bass
import concourse.mybir as mybir
from concourse.bass_utils import run_bass_kernel_spmd

N_CORES = 8
IN_SIZE = 4096
BATCH = 8192
P = 128                                # SBUF partitions
ROWS_PER_CORE = IN_SIZE // N_CORES     # 512 rows of xT per core
N_PBLK = ROWS_PER_CORE // P            # 4 partition blocks per core
AUG = 2                                # leading [w, b] columns per row
W = AUG + BATCH                        # augmented row width

# test.py hooks: set TRACE=True before calling kernel() to capture an NTFF
# profile; the BassKernelResults land in LAST_RESULTS.
TRACE = False
LAST_RESULTS = None

_cached_nc = None


def _build():
    f32 = mybir.dt.float32
    nc = bass.Bass(trn_type="TRN2")
    xt = nc.dram_tensor("xt", [ROWS_PER_CORE, W], f32, kind="ExternalInput")
    yt = nc.dram_tensor("yt", [ROWS_PER_CORE, BATCH], f32, kind="ExternalOutput")

    with (
        nc.sbuf_tensor("t0", [P, W], f32) as t0,
        nc.sbuf_tensor("t1", [P, W], f32) as t1,
        nc.sbuf_tensor("t2", [P, W], f32) as t2,
        nc.sbuf_tensor("t3", [P, W], f32) as t3,
        nc.semaphore("in_sp") as in_sp,
        nc.semaphore("in_act") as in_act,
        nc.semaphore("dve_done") as dve_done,
        nc.semaphore("out_sp") as out_sp,
        nc.semaphore("out_act") as out_act,
        nc.Block() as block,
    ):
        tiles = [t0, t1, t2, t3]
        rows = [slice(k * P, (k + 1) * P) for k in range(N_PBLK)]

        # Tiles 0, 2 move on the SP ring; tiles 1, 3 on the ACT ring.
        @block.sync
        def _(sync):
            sync.dma_start(t0[:], xt[rows[0], :]).then_inc(in_sp, 16)
            sync.dma_start(t2[:], xt[rows[2], :]).then_inc(in_sp, 16)
            sync.wait_ge(dve_done, 1)
            sync.dma_start(yt[rows[0], :], t0[:, AUG:]).then_inc(out_sp, 16)
            sync.wait_ge(dve_done, 3)
            sync.dma_start(yt[rows[2], :], t2[:, AUG:]).then_inc(out_sp, 16)
            sync.wait_ge(out_sp, 32)

        @block.scalar
        def _(scalar):
            scalar.dma_start(t1[:], xt[rows[1], :]).then_inc(in_act, 16)
            scalar.dma_start(t3[:], xt[rows[3], :]).then_inc(in_act, 16)
            scalar.wait_ge(dve_done, 2)
            scalar.dma_start(yt[rows[1], :], t1[:, AUG:]).then_inc(out_act, 16)
            scalar.wait_ge(dve_done, 4)
            scalar.dma_start(yt[rows[3], :], t3[:, AUG:]).then_inc(out_act, 16)
            scalar.wait_ge(out_act, 32)

        @block.vector
        def _(vector):
            waits = [(in_sp, 16), (in_act, 16), (in_sp, 32), (in_act, 32)]
            for k, t in enumerate(tiles):
                sem, val = waits[k]
                vector.wait_ge(sem, val)
                vector.tensor_scalar(
                    out=t[:, AUG:],
                    in0=t[:, AUG:],
                    scalar1=t[:, 0:1],
                    scalar2=t[:, 1:2],
                    op0=mybir.AluOpType.mult,
                    op1=mybir.AluOpType.add,
                ).then_inc(dve_done, 1)

    return nc


def kernel(x, weight, bias):
    global LAST_RESULTS, _cached_nc
    x = np.ascontiguousarray(np.asarray(x), dtype=np.float32)
    weight = np.ascontiguousarray(np.asarray(weight), dtype=np.float32)
    bias = np.ascontiguousarray(np.asarray(bias), dtype=np.float32)
    assert x.shape == (BATCH, IN_SIZE)

    # Build the augmented transposed input: row r of xta is
    # [weight[r], bias[r], x[0, r], x[1, r], ..., x[BATCH-1, r]].
    xta = np.empty((IN_SIZE, W), dtype=np.float32)
    xta[:, 0] = weight
    xta[:, 1] = bias
    xta[:, AUG:] = x.T

    if _cached_nc is None:
        _cached_nc = _build()
    nc = _cached_nc

    in_maps = []
    for c in range(N_CORES):
        r0 = c * ROWS_PER_CORE
        in_maps.append({"xt": xta[r0:r0 + ROWS_PER_CORE]})

    res = run_bass_kernel_spmd(
        nc, in_maps, core_ids=list(range(N_CORES)), trace=TRACE
    )
    LAST_RESULTS = res
    yT = np.concatenate([r["yt"] for r in res.results], axis=0)  # [IN_SIZE, BATCH]
    return np.ascontiguousarray(yT.T)


# revision 10
# speedup vs baseline: 1.8215x; 1.8215x over previous
"""DiagLinear kernel for 8 TRN2 NeuronCores.

Computes y = x * weight + bias  (weight/bias broadcast over the batch dim).

Strategy: transpose x on the host to xT [IN_SIZE, BATCH] and shard xT's rows
(the in_size dim) across the 8 cores. With in_size on the SBUF partition
axis, weight/bias become per-partition scalars, so the whole elementwise
computation is a single fused DVE tensor_scalar op per tile:
    out = (x * w) + b          (fp32, 2x perf mode)
which keeps the kernel firmly DMA-bound (the memory roofline for this
problem: 2 x 16.78 MB of HBM traffic per core at ~358 GB/s ~= 94 us).

Each row of the per-core input is augmented on the host with 2 leading
columns holding that row's weight and bias values, so every SBUF tile is
self-contained: the fused op reads its per-partition scalars from columns
0/1 of the tile it just loaded. The kernel is raw Bass (no Tile) with a
fully static schedule: 4 tiles of [128, 2+8192], loads and stores split
across the two HWDGE rings (SP and ACT sequencers), DVE compute chained
behind each load via semaphores.
"""

import numpy as np

import concourse.bass as bass
import concourse.mybir as mybir
from concourse.bass_utils import run_bass_kernel_spmd

N_CORES = 8
IN_SIZE = 4096
BATCH = 8192
P = 128                                # SBUF partitions
ROWS_PER_CORE = IN_SIZE // N_CORES     # 512 rows of xT per core
N_PBLK = ROWS_PER_CORE // P            # 4 partition blocks per core
AUG = 2                                # leading [w, b] columns per row
W = AUG + BATCH                        # augmented row width

# test.py hooks: set TRACE=True before calling kernel() to capture an NTFF
# profile; the BassKernelResults land in LAST_RESULTS.
TRACE = False
LAST_RESULTS = None

_cached_nc = None


def _build():
    f32 = mybir.dt.float32
    nc = bass.Bass(trn_type="TRN2", enable_partition_id=False)
    xt = nc.dram_tensor("xt", [ROWS_PER_CORE, W], f32, kind="ExternalInput")
    yt = nc.dram_tensor("yt", [ROWS_PER_CORE, BATCH], f32, kind="ExternalOutput")

    with (
        nc.sbuf_tensor("t0", [P, W], f32) as t0,
        nc.sbuf_tensor("t1", [P, W], f32) as t1,
        nc.sbuf_tensor("t2", [P, W], f32) as t2,
        nc.sbuf_tensor("t3", [P, W], f32) as t3,
        nc.semaphore("in_sp") as in_sp,
        nc.semaphore("in_act") as in_act,
        nc.semaphore("dve_done") as dve_done,
        nc.semaphore("out_sp") as out_sp,
        nc.semaphore("out_act") as out_act,
        nc.Block() as block,
    ):
        tiles = [t0, t1, t2, t3]
        rows = [slice(k * P, (k + 1) * P) for k in range(N_PBLK)]

        # Tiles 0, 2 move on the SP ring; tiles 1, 3 on the ACT ring.
        @block.sync
        def _(sync):
            sync.dma_start(t0[:], xt[rows[0], :]).then_inc(in_sp, 16)
            sync.dma_start(t2[:], xt[rows[2], :]).then_inc(in_sp, 16)
            sync.wait_ge(dve_done, 1)
            sync.dma_start(yt[rows[0], :], t0[:, AUG:]).then_inc(out_sp, 16)
            sync.wait_ge(dve_done, 3)
            sync.dma_start(yt[rows[2], :], t2[:, AUG:]).then_inc(out_sp, 16)

        @block.scalar
        def _(scalar):
            scalar.dma_start(t1[:], xt[rows[1], :]).then_inc(in_act, 16)
            scalar.dma_start(t3[:], xt[rows[3], :]).then_inc(in_act, 16)
            scalar.wait_ge(dve_done, 2)
            scalar.dma_start(yt[rows[1], :], t1[:, AUG:]).then_inc(out_act, 16)
            scalar.wait_ge(dve_done, 4)
            scalar.dma_start(yt[rows[3], :], t3[:, AUG:]).then_inc(out_act, 16)

        @block.vector
        def _(vector):
            waits = [(in_sp, 16), (in_act, 16), (in_sp, 32), (in_act, 32)]
            for k, t in enumerate(tiles):
                sem, val = waits[k]
                vector.wait_ge(sem, val)
                vector.tensor_scalar(
                    out=t[:, AUG:],
                    in0=t[:, AUG:],
                    scalar1=t[:, 0:1],
                    scalar2=t[:, 1:2],
                    op0=mybir.AluOpType.mult,
                    op1=mybir.AluOpType.add,
                ).then_inc(dve_done, 1)

    return nc


def kernel(x, weight, bias):
    global LAST_RESULTS, _cached_nc
    x = np.ascontiguousarray(np.asarray(x), dtype=np.float32)
    weight = np.ascontiguousarray(np.asarray(weight), dtype=np.float32)
    bias = np.ascontiguousarray(np.asarray(bias), dtype=np.float32)
    assert x.shape == (BATCH, IN_SIZE)

    # Build the augmented transposed input: row r of xta is
    # [weight[r], bias[r], x[0, r], x[1, r], ..., x[BATCH-1, r]].
    xta = np.empty((IN_SIZE, W), dtype=np.float32)
    xta[:, 0] = weight
    xta[:, 1] = bias
    xta[:, AUG:] = x.T

    if _cached_nc is None:
        _cached_nc = _build()
    nc = _cached_nc

    in_maps = []
    for c in range(N_CORES):
        r0 = c * ROWS_PER_CORE
        in_maps.append({"xt": xta[r0:r0 + ROWS_PER_CORE]})

    res = run_bass_kernel_spmd(
        nc, in_maps, core_ids=list(range(N_CORES)), trace=TRACE
    )
    LAST_RESULTS = res
    yT = np.concatenate([r["yt"] for r in res.results], axis=0)  # [IN_SIZE, BATCH]
    return np.ascontiguousarray(yT.T)
